# revision 11
# baseline (speedup 1.0000x reference)
"""3-layer GAT on 8 Trainium2 NeuronCores (Bass/Tile).

Edge-sharded by destination range:
  - Nodes split into 8 contiguous ranges (one per core); each core owns the
    softmax + aggregation for its destination nodes.
  - Layer 1's packed per-node table [h | a_src] (c-major feature order) is
    computed REPLICATED on every core (x is cheap to re-multiply at bf16),
    killing the first AllGather.  Layers 2/3 AllGather their tables in 4
    row-chunks issued as destination-tile groups complete, overlapping the
    collective with the remaining aggregation compute.
  - Edges (with self loops) are bucketed per core into 128-dst tiles x
    128-edge chunks; chunk structure (incl. lo/hi int16-index table halves)
    is made identical across cores so one SPMD instruction stream fits all.
  - Per 8-chunk super-batch the kernel dma_gathers source rows + dest
    attention rows, computes w = exp(leaky_relu(a_src+a_dst)) (softmax
    shift-invariance removes the segment-max pass at these value ranges),
    scales messages by w, and segment-sums with matmuls against one-hot
    membership matrices, keeping numerator and denominator together in
    PSUM.  The membership matrix mt is built ON DEVICE per super-batch with
    a single is_equal broadcast op against host-packed dst-slot columns
    (the transposed variant mtT, needed for the per-edge a_dst matmul,
    stays host-built in meta).  The per-tile epilogue divides, applies
    bias/relu, and feeds the next layer's matmul whose rhs
    [W | W@att_src | W@att_dst] also emits the next attention scores.
"""

import numpy as np
import ml_dtypes

N = 50000
E = 800000
IN_C = 128
HID = 32
OUT_C = 40
HEADS = 8
NEG_SLOPE = 0.2
NCORES = 8

_BF16 = ml_dtypes.bfloat16

KSUP = 8  # chunks per gather super-batch (1024 idx = dma_gather limit)
# AllGather row-chunk tile-group sizes (sum = ntiles = 49). Front-loaded so
# early chunks (issued earliest) carry the bytes and the tail chunk - the
# only one with no compute left to hide under - is tiny.
AG_SIZES = [24, 14, 8, 3]


def _cmajor_perm(heads, ch):
    f_new = np.arange(heads * ch)
    return (f_new % heads) * ch + f_new // heads  # perm[new] = old


def _attn_cols(w, att):
    heads, ch = att.shape
    return np.einsum("khc,hc->kh", w.reshape(-1, heads, ch), att).astype(np.float32)


def _prep_weights(W1, as1, ad1, b1, W2, as2, ad2, b2, W3, as3, ad3, b3):
    W1 = np.asarray(W1, np.float32)
    W2 = np.asarray(W2, np.float32)
    W3 = np.asarray(W3, np.float32)
    perm = _cmajor_perm(HEADS, HID)

    rhs1 = np.concatenate(
        [W1[:, perm], _attn_cols(W1, np.asarray(as1, np.float32)),
         _attn_cols(W1, np.asarray(ad1, np.float32))], axis=1).astype(_BF16)
    W2r = W2[perm, :]
    rhs2 = np.concatenate(
        [W2r[:, perm], _attn_cols(W2r, np.asarray(as2, np.float32)),
         _attn_cols(W2r, np.asarray(ad2, np.float32))], axis=1).astype(np.float32)
    W3r = W3[perm, :]
    as3p = (W3r @ np.asarray(as3, np.float32)[0]).reshape(-1, 1)
    ad3p = (W3r @ np.asarray(ad3, np.float32)[0]).reshape(-1, 1)
    rhs3 = np.concatenate([W3r, as3p, ad3p], axis=1).astype(np.float32)

    def bcast(b):
        return np.tile(np.asarray(b, np.float32)[None, :], (128, 1))

    return (rhs1, rhs2, rhs3,
            bcast(np.asarray(b1, np.float32)[perm]),
            bcast(np.asarray(b2, np.float32)[perm]),
            bcast(np.asarray(b3, np.float32)))


def _ag_groups(ntiles):
    """Tile-index ranges of the AllGather row-chunk groups."""
    sizes = AG_SIZES if sum(AG_SIZES) == ntiles else [ntiles]
    groups = []
    t0 = 0
    for s in sizes:
        groups.append((t0, t0 + s))
        t0 += s
    return groups


def _prep_graph(edge_index):
    """Slot edges into the SPMD-uniform (tile, section, chunk) grid.

    Self loops are NOT materialized as edges; their contribution is
    injected per destination tile in the aggregation prologue matmul.
    Edges are tagged by the AllGather group of their SOURCE row, since
    each group is a separate table tensor (single-writer collectives);
    gather indices are rows within the group tensor.
    """
    src = np.asarray(edge_index[0]).astype(np.int64)
    dst = np.asarray(edge_index[1]).astype(np.int64)

    npc = N // NCORES
    ntiles = (npc + 127) // 128
    nmax = ntiles * 128

    groups = _ag_groups(ntiles)
    G = len(groups)
    grp_of_tile = np.zeros(ntiles, np.int64)
    grp_a = np.zeros(G, np.int64)
    grp_sz = np.zeros(G, np.int64)
    for gi, (a, b) in enumerate(groups):
        grp_of_tile[a:b] = gi
        grp_a[gi] = a
        grp_sz[gi] = b - a

    core_of = dst // npc
    d_loc = dst - core_of * npc
    tile_of = d_loc // 128
    s_core = src // npc
    s_loc = src - s_core * npc
    s_tag = grp_of_tile[s_loc // 128]
    # row within the group tensor [NCORES * grp_sz * 128]
    s_row = (s_core * grp_sz[s_tag] + (s_loc // 128 - grp_a[s_tag])) * 128 \
        + s_loc % 128
    assert s_row.max() < 32768

    cnt = np.zeros((NCORES, ntiles, G), np.int64)
    np.add.at(cnt, (core_of, tile_of, s_tag), 1)
    sec_cpt = np.ceil(cnt / 128).astype(np.int64).max(axis=0)  # [ntiles, G]
    sec_cpt[:, 0] = np.maximum(sec_cpt[:, 0], 1)

    total = int(sec_cpt.sum())
    pad = (-total) % KSUP
    sec_cpt[-1, -1] += pad
    total += pad
    nsup = total // KSUP

    # pair adjacent tiles so same-tag sections are contiguous: longer
    # dma_gather runs (fewer gathers) at the cost of two concurrently
    # open PSUM accumulators
    sec_order = []
    for t0 in range(0, ntiles, 2):
        ts = [t0] if t0 + 1 >= ntiles else [t0, t0 + 1]
        for gtag in range(G):
            sec_order += [(t, gtag) for t in ts]
    tile_of_chunk = []
    tag_of_chunk = []
    sec_base = np.zeros((ntiles, G), np.int64)
    off = 0
    for (t, tg) in sec_order:
        n = int(sec_cpt[t, tg])
        sec_base[t, tg] = off
        tile_of_chunk += [t] * n
        tag_of_chunk += [tg] * n
        off += n
    tile_of_chunk = np.array(tile_of_chunk)
    tag_of_chunk = np.array(tag_of_chunk)

    MW = 64 + KSUP + KSUP * 128  # idx | dst-slot cols (bf16) | mtT
    src_w = np.zeros((NCORES, 128, total * 8), np.int16)
    meta = np.zeros((NCORES, nsup, 128, MW), np.int16)

    order = np.lexsort((src, s_tag, tile_of, core_of))
    src_o = s_row[order]
    dst_o = d_loc[order]
    core_o = core_of[order]
    tile_o = tile_of[order]
    tag_o = s_tag[order]

    for k in range(NCORES):
        m = core_o == k
        t = tile_o[m]
        tg = tag_o[m]
        sr = src_o[m]
        dl = dst_o[m]
        key = t * G + tg
        cnts = np.bincount(key, minlength=ntiles * G)
        st = np.zeros(ntiles * G, np.int64)
        st[1:] = np.cumsum(cnts)[:-1]
        pos = np.arange(len(t)) - st[key]
        q = sec_base[t, tg] + pos // 128
        p = pos % 128
        col = q * 8 + p // 16
        row = p % 16
        for c in range(8):
            src_w[k, row + 16 * c, col] = sr
        D = np.full((total, 128), 255, np.int16)
        D[q, p] = (dl % 128).astype(np.int16)
        # dst-slot columns, bf16 bits: Dcol[s, p, kk] = D[s*8+kk, p]
        Dcol = D.reshape(nsup, KSUP, 128).transpose(0, 2, 1).astype(_BF16)
        meta[k, :, :, 64:64 + KSUP] = Dcol.view(np.int16)
        # host-built transposed one-hot: mtT[chunk][j, p] = (dl[p] == j)
        oneh = (D[:, :, None] == np.arange(128, dtype=np.int16)[None, None, :]
                ).astype(_BF16)
        mtTs = oneh.transpose(0, 2, 1).reshape(
            nsup, KSUP, 128, 128).transpose(0, 2, 1, 3).reshape(
            nsup, 128, KSUP * 128)
        meta[k, :, :, 64 + KSUP:] = mtTs.view(np.int16)

    runs = []  # (sup, chunk_lo, chunk_hi, tag)
    for s in range(nsup):
        q0 = s * KSUP
        r0 = q0
        for q in range(q0 + 1, q0 + KSUP + 1):
            if q == q0 + KSUP or tag_of_chunk[q] != tag_of_chunk[r0]:
                runs.append((s, r0, q, int(tag_of_chunk[r0])))
                r0 = q

    for k in range(NCORES):
        meta[k, :, :, :64] = src_w[k].reshape(128, nsup, 64).transpose(
            1, 0, 2)

    return dict(
        meta=meta,
        tile_of_chunk=tile_of_chunk, runs=runs, nsup=nsup, total=total,
        ntiles=ntiles, nmax=nmax, npc=npc,
    )


def _build_bass(g, repeat=1):
    import concourse.bacc as bacc
    import concourse.mybir as mybir
    import concourse.tile as tile
    from concourse.masks import make_identity

    dt = mybir.dt
    Alu = mybir.AluOpType
    Act = mybir.ActivationFunctionType

    ntiles, nmax, nsup, total = g["ntiles"], g["nmax"], g["nsup"], g["total"]
    tile_of_chunk = g["tile_of_chunk"]
    H2 = HEADS * HID  # 256
    PACK = H2 + 2 * HEADS  # 272 psum width: h + a_src + a_dst
    TW = 384  # table row width (768B)
    TW3 = 128  # layer-3 / a_dst table row width (256B)
    GW = H2 + HEADS  # 264 useful gathered cols
    GW3 = OUT_C + 1  # 41
    GTILES = NCORES * ntiles  # replicated layer-1 tiles
    MW = 64 + KSUP + KSUP * 128

    first_chunk = {}
    last_chunk = {}
    for q, t in enumerate(tile_of_chunk):
        first_chunk.setdefault(int(t), q)
        last_chunk[int(t)] = q
    runs_by_sup = {}
    for (s, a, b, tag) in g["runs"]:
        runs_by_sup.setdefault(s, []).append((a, b, tag))

    groups = _ag_groups(ntiles)
    grp_of_tile = {}
    for gi, (a, b) in enumerate(groups):
        for t in range(a, b):
            grp_of_tile[t] = gi

    nc = bacc.Bacc("TRN2", target_bir_lowering=False, debug=False,
                   num_devices=NCORES, num_swdge_queues=4)

    xTf = nc.dram_tensor("xTf", [IN_C, GTILES * 128], dt.bfloat16,
                         kind="ExternalInput")  # full graph, replicated
    xT = nc.dram_tensor("xT", [IN_C, nmax], dt.bfloat16,
                        kind="ExternalInput")  # own slice
    rhs1 = nc.dram_tensor("rhs1", [IN_C, PACK], dt.bfloat16,
                          kind="ExternalInput")
    rhs2 = nc.dram_tensor("rhs2", [H2, PACK], dt.float32, kind="ExternalInput")
    rhs3 = nc.dram_tensor("rhs3", [H2, OUT_C + 2], dt.float32,
                          kind="ExternalInput")
    b1r = nc.dram_tensor("b1r", [128, H2], dt.float32, kind="ExternalInput")
    b2r = nc.dram_tensor("b2r", [128, H2], dt.float32, kind="ExternalInput")
    b3r = nc.dram_tensor("b3r", [128, OUT_C], dt.float32, kind="ExternalInput")
    meta_in = nc.dram_tensor("meta", [nsup, 128, MW],
                             dt.int16, kind="ExternalInput")
    out = nc.dram_tensor("out", [nmax, OUT_C], dt.float32,
                         kind="ExternalOutput")

    import os
    _SIM = bool(int(os.environ.get("GAT_SIM", "0")))

    with tile.TileContext(nc) as tc:
        with (
            tc.tile_pool(name="const", bufs=1) as constp,
            tc.tile_pool(name="sbuf", bufs=6) as sbuf,
            tc.tile_pool(name="gbuf", bufs=8) as gbuf,
            tc.tile_pool(name="mtb", bufs=3) as mtb,
            tc.tile_pool(name="xb", bufs=3) as xb,
            tc.tile_pool(name="epil", bufs=2) as epil,
            tc.tile_pool(name="sfbuf", bufs=4) as sfbuf,
            tc.tile_pool(name="psum_seg", bufs=3, space="PSUM") as psum_seg,
            tc.tile_pool(name="psum_h", bufs=2, space="PSUM") as psum_h,
            tc.tile_pool(name="psum_tp", bufs=1, space="PSUM") as psum_tp,
            tc.tile_pool(name="psum_w", bufs=2, space="PSUM") as psum_w,
            tc.tile_pool(name="dram", bufs=1, space="DRAM") as dram,
        ):
            # ---- constants ----
            xT_s = constp.tile([IN_C, nmax], dt.bfloat16)
            nc.sync.dma_start(out=xT_s[:], in_=xT[:])
            rhs1_s = constp.tile([IN_C, PACK], dt.bfloat16)
            nc.sync.dma_start(out=rhs1_s[:], in_=rhs1[:])
            rhs2_s = constp.tile([128, 2 * PACK], dt.float32)
            nc.sync.dma_start(
                out=rhs2_s[:].rearrange("p (k f) -> p k f", k=2),
                in_=rhs2[:].rearrange("(k p) f -> p k f", p=128))
            rhs3_s = constp.tile([128, 2 * (OUT_C + 2)], dt.float32)
            nc.sync.dma_start(
                out=rhs3_s[:].rearrange("p (k f) -> p k f", k=2),
                in_=rhs3[:].rearrange("(k p) f -> p k f", p=128))
            b1_s = constp.tile([128, H2], dt.float32)
            nc.sync.dma_start(out=b1_s[:], in_=b1r[:])
            b2_s = constp.tile([128, H2], dt.float32)
            nc.sync.dma_start(out=b2_s[:], in_=b2r[:])
            b3_s = constp.tile([128, OUT_C], dt.float32)
            nc.sync.dma_start(out=b3_s[:], in_=b3r[:])
            ident = constp.tile([128, 128], dt.float32)
            make_identity(nc, ident[:])
            identb = constp.tile([128, 128], dt.bfloat16)
            make_identity(nc, identb[:])
            zpad = constp.tile([128, TW3 - 1], dt.bfloat16)
            nc.vector.memset(zpad[:], 0.0)
            # tiled iota row: iota_row[p, kk*128 + j] = j  (bf16, exact)
            iota_row = constp.tile([128, KSUP * 128], dt.bfloat16)
            nc.gpsimd.iota(iota_row[:], pattern=[[0, KSUP], [1, 128]],
                           channel_multiplier=0,
                           allow_small_or_imprecise_dtypes=True)

            # per-layer destination-attention tables (tile-local, SBUF)
            adstA = constp.tile([128, ntiles * HEADS], dt.bfloat16)
            adstB = constp.tile([128, ntiles * HEADS], dt.bfloat16)
            adstC = constp.tile([128, ntiles], dt.bfloat16)

            # ---- DRAM temporaries ----
            # per-AG-chunk local slices of the layer-2/3 tables
            loc2g = [dram.tile([(b - a) * 128, TW], dt.bfloat16,
                               name=f"loc2g{i}")
                     for i, (a, b) in enumerate(groups)]
            loc3g = [dram.tile([(b - a) * 128, TW3], dt.bfloat16,
                               name=f"loc3g{i}")
                     for i, (a, b) in enumerate(groups)]
            # per-layer self-loop contributions [w*h | w], injected as the
            # accumulation-starting matmul of each destination tile
            selfA = dram.tile([nmax, GW], dt.bfloat16)
            selfB = dram.tile([nmax, GW], dt.bfloat16)
            selfC = dram.tile([nmax, GW3], dt.bfloat16)

            # zero never-written pad columns once (NaN hygiene)
            for gi, (a, b) in enumerate(groups):
                nt = b - a
                for buf, c0 in ((loc2g[gi], GW), (loc3g[gi], GW3)):
                    w = buf.shape[1] - c0
                    nc.sync.dma_start(
                        out=buf[:].rearrange("(t p) w -> p t w", p=128)[
                            :, :, c0:],
                        in_=zpad[:, :w].unsqueeze(1).to_broadcast(
                            [128, nt, w]))

            def loc2_write(t):
                gi = grp_of_tile[t]
                return loc2g[gi], (t - groups[gi][0]) * 128

            def loc3_write(t):
                gi = grp_of_tile[t]
                return loc3g[gi], (t - groups[gi][0]) * 128

            def extras12(ps, pk, adst_next, self_next, t):
                """a_dst table entry + self-loop row for own tile t.

                ps: PSUM [128, PACK]; pk: packed bf16 [128, GW] of same."""
                nc.scalar.copy(
                    out=adst_next[:, t * HEADS:(t + 1) * HEADS],
                    in_=ps[:, GW:GW + HEADS])
                # self-loop term for the next aggregation: [w*h | w],
                # w = exp(leaky(a_src + a_dst)) of the node itself
                ws = epil.tile([128, HEADS], dt.bfloat16, tag="ws")
                nc.vector.tensor_tensor(
                    out=ws[:], in0=pk[:, H2:GW],
                    in1=adst_next[:, t * HEADS:(t + 1) * HEADS], op=Alu.add)
                nc.vector.scalar_tensor_tensor(
                    out=ws[:], in0=ws[:], scalar=NEG_SLOPE, in1=ws[:],
                    op0=Alu.mult, op1=Alu.max)
                nc.scalar.activation(out=ws[:], in_=ws[:], func=Act.Exp)
                sr = epil.tile([128, GW], dt.bfloat16, tag="selfr")
                nc.vector.tensor_tensor(
                    out=sr[:, :H2].rearrange("p (c h) -> p c h", h=HEADS),
                    in0=ps[:, :H2].rearrange("p (c h) -> p c h", h=HEADS),
                    in1=ws[:].unsqueeze(1).to_broadcast([128, HID, HEADS]),
                    op=Alu.mult)
                nc.vector.tensor_copy(sr[:, H2:GW], ws[:])
                nc.sync.dma_start(
                    out=self_next[t * 128:(t + 1) * 128, :GW], in_=sr[:])

            def pack12(ps, adst_next, self_next, t):
                """Layer-1/2 epilogue tail for own tile t: write local table
                slice + extras."""
                pk = epil.tile([128, GW], dt.bfloat16, tag="pack")
                nc.scalar.copy(out=pk[:], in_=ps[:, :GW])
                buf, r0 = loc2_write(t)
                nc.sync.dma_start(out=buf[r0:r0 + 128, :GW], in_=pk[:])
                extras12(ps, pk, adst_next, self_next, t)

            def pack3(ps, t):
                pk = epil.tile([128, GW3], dt.bfloat16, tag="pack")
                nc.scalar.copy(out=pk[:], in_=ps[:, :GW3])
                buf, r0 = loc3_write(t)
                nc.sync.dma_start(out=buf[r0:r0 + 128, :GW3], in_=pk[:])
                nc.scalar.copy(out=adstC[:, t:t + 1],
                               in_=ps[:, GW3:GW3 + 1])
                ws = epil.tile([128, 1], dt.float32, tag="ws3")
                nc.vector.tensor_tensor(
                    out=ws[:], in0=pk[:, OUT_C:OUT_C + 1],
                    in1=adstC[:, t:t + 1], op=Alu.add)
                nc.vector.scalar_tensor_tensor(
                    out=ws[:], in0=ws[:], scalar=NEG_SLOPE, in1=ws[:],
                    op0=Alu.mult, op1=Alu.max)
                nc.scalar.activation(out=ws[:], in_=ws[:], func=Act.Exp)
                sr = epil.tile([128, GW3], dt.bfloat16, tag="selfr3")
                nc.vector.tensor_scalar(
                    out=sr[:, :OUT_C], in0=ps[:, :OUT_C],
                    scalar1=ws[:, :1], scalar2=None, op0=Alu.mult)
                nc.vector.tensor_copy(sr[:, OUT_C:GW3], ws[:])
                nc.sync.dma_start(
                    out=selfC[t * 128:(t + 1) * 128, :GW3], in_=sr[:])

            def h1_phase(tab1g):
                # pass 1: own-slice extras (a_dst table + self rows)
                for t in range(ntiles):
                    ps = psum_h.tile([128, PACK], dt.float32, tag="hps")
                    nc.tensor.matmul(
                        ps[:], lhsT=xT_s[:, t * 128:(t + 1) * 128],
                        rhs=rhs1_s[:], start=True, stop=True)
                    pk = epil.tile([128, GW], dt.bfloat16, tag="pack")
                    nc.scalar.copy(out=pk[:], in_=ps[:, :GW])
                    extras12(ps, pk, adstA, selfA, t)
                # pass 2: replicated full-graph table build (no AllGather)
                XS = 8  # tiles per x super-load
                for ss in range((GTILES + XS - 1) // XS):
                    t0 = ss * XS
                    nt = min(XS, GTILES - t0)
                    xt = xb.tile([IN_C, XS * 128], dt.bfloat16, tag="xt")
                    nc.sync.dma_start(
                        out=xt[:, :nt * 128],
                        in_=xTf[:, t0 * 128:(t0 + nt) * 128])
                    for j in range(nt):
                        gt_i = t0 + j
                        k, t = gt_i // ntiles, gt_i % ntiles
                        gi = grp_of_tile[t]
                        a, b = groups[gi]
                        r0 = (k * (b - a) + (t - a)) * 128
                        ps = psum_h.tile([128, PACK], dt.float32, tag="hps")
                        nc.tensor.matmul(
                            ps[:, :GW], lhsT=xt[:, j * 128:(j + 1) * 128],
                            rhs=rhs1_s[:, :GW], start=True, stop=True)
                        pk = epil.tile([128, GW], dt.bfloat16, tag="pack")
                        nc.scalar.copy(out=pk[:], in_=ps[:, :GW])
                        nc.sync.dma_start(
                            out=tab1g[gi][r0:r0 + 128, :GW], in_=pk[:])

            def allgather_chunk(locg, tabg, gi):
                if _SIM:
                    # TimelineSim can't model collectives: stand in a local
                    # DMA copy so deps stay realistic.
                    nc.sync.dma_start(out=tabg[gi][:locg[gi].shape[0]],
                                      in_=locg[gi][:])
                    return
                nc.gpsimd.collective_compute(
                    "AllGather", Alu.bypass,
                    replica_groups=[list(range(NCORES))],
                    ins=[locg[gi][:].opt()], outs=[tabg[gi][:].opt()])

            def epilogue12(t, ps, rhs_next_s, b_s, layer, adst_next,
                           self_next):
                deneps = epil.tile([128, HEADS], dt.float32, tag="deneps")
                nc.vector.tensor_scalar_add(deneps[:], ps[:, H2:H2 + HEADS],
                                            1e-16)
                recip = epil.tile([128, HEADS], dt.float32, tag="recip")
                nc.vector.reciprocal(recip[:], deneps[:])
                act = epil.tile([128, H2], dt.float32, tag="act")
                nc.vector.tensor_tensor(
                    out=act[:].rearrange("p (c h) -> p c h", h=HEADS),
                    in0=ps[:, :H2].rearrange("p (c h) -> p c h", h=HEADS),
                    in1=recip[:].unsqueeze(1).to_broadcast([128, HID, HEADS]),
                    op=Alu.mult)
                nc.vector.tensor_add(out=act[:], in0=act[:], in1=b_s[:])
                nc.scalar.activation(out=act[:], in_=act[:], func=Act.Relu)
                w = PACK if layer == 1 else OUT_C + 2
                hps = psum_h.tile([128, PACK], dt.float32, tag="hps")
                for kc in range(2):
                    tp = psum_tp.tile([128, 128], dt.float32, tag="tp")
                    nc.tensor.transpose(
                        out=tp[:], in_=act[:, kc * 128:(kc + 1) * 128],
                        identity=ident[:])
                    aT = epil.tile([128, 128], dt.float32, tag="aT")
                    nc.scalar.copy(out=aT[:], in_=tp[:])
                    nc.tensor.matmul(
                        hps[:, :w], lhsT=aT[:],
                        rhs=rhs_next_s[:, kc * w:(kc + 1) * w],
                        start=(kc == 0), stop=(kc == 1))
                if layer == 1:
                    pack12(hps, adst_next, self_next, t)
                else:
                    pack3(hps, t)

            def epilogue3(t, ps):
                deneps = epil.tile([128, 1], dt.float32, tag="deneps3")
                nc.vector.tensor_scalar_add(deneps[:], ps[:, OUT_C:OUT_C + 1],
                                            1e-16)
                recip = epil.tile([128, 1], dt.float32, tag="recip3")
                nc.vector.reciprocal(recip[:], deneps[:])
                o3 = epil.tile([128, OUT_C], dt.float32, tag="o3")
                nc.vector.tensor_scalar(
                    out=o3[:], in0=ps[:, :OUT_C], scalar1=recip[:, :1],
                    scalar2=None, op0=Alu.mult)
                nc.vector.tensor_add(out=o3[:], in0=o3[:], in1=b3_s[:])
                mneg = epil.tile([128, 1], dt.float32, tag="mneg")
                nc.vector.tensor_reduce(
                    out=mneg[:], in_=o3[:], axis=mybir.AxisListType.X,
                    op=Alu.max, negate=True)
                es = epil.tile([128, OUT_C], dt.float32, tag="es")
                ssum = epil.tile([128, 1], dt.float32, tag="ssum")
                nc.scalar.activation(out=es[:], in_=o3[:], func=Act.Exp,
                                     bias=mneg[:, :1], accum_out=ssum[:, :1])
                lse = epil.tile([128, 1], dt.float32, tag="lse")
                nc.scalar.activation(out=lse[:], in_=ssum[:], func=Act.Ln)
                fin = epil.tile([128, OUT_C], dt.float32, tag="fin")
                nc.vector.tensor_scalar(
                    out=fin[:], in0=o3[:], scalar1=mneg[:, :1],
                    scalar2=lse[:, :1], op0=Alu.add, op1=Alu.subtract)
                nc.sync.dma_start(out=out[t * 128:(t + 1) * 128, :], in_=fin[:])

            def aggregate(layer, tabg, adst_tab, rhs_next_s, b_s,
                          adst_next, self_next, selfbuf, ag_next=None):
                """ag_next: (loc_list, next_table_list) to AllGather in
                chunks as tile groups complete (layer 1 -> tab2,
                layer 2 -> tab3)."""
                if layer == 3:
                    gw, nfeat, nh, tw = GW3, OUT_C, 1, TW3
                else:
                    gw, nfeat, nh, tw = GW, H2, HEADS, TW

                ps_open = {}
                sf_map = {}
                pend = []
                qctr = [0]

                def seg_stage(sup, gt, mt):
                    for kk in range(KSUP):
                        q = sup * KSUP + kk
                        t = int(tile_of_chunk[q])
                        if q == first_chunk[t]:
                            ps_new = psum_seg.tile([128, GW], dt.float32,
                                                   tag="segps")
                            ps_open[t] = ps_new
                            nc.tensor.matmul(
                                ps_new[:, :gw], lhsT=identb[:],
                                rhs=sf_map.pop(t)[:], start=True,
                                stop=False)
                        nc.tensor.matmul(
                            ps_open[t][:, :gw],
                            lhsT=mt[:, kk * 128:(kk + 1) * 128],
                            rhs=gt[:, kk, :gw],
                            start=False,
                            stop=(q == last_chunk[t]))
                        if q == last_chunk[t]:
                            ps_done = ps_open.pop(t)
                            if layer == 3:
                                epilogue3(t, ps_done)
                            else:
                                epilogue12(t, ps_done, rhs_next_s, b_s,
                                           layer, adst_next, self_next)
                            if ag_next is not None:
                                locs, ntabg = ag_next
                                gi = grp_of_tile[t]
                                if t == groups[gi][1] - 1:
                                    allgather_chunk(locs, ntabg, gi)

                for sup in range(nsup):
                    mtile = sbuf.tile([128, MW], dt.int16, tag="meta")
                    nc.sync.dma_start(out=mtile[:], in_=meta_in[sup])
                    dcol = mtile[:, 64:64 + KSUP].bitcast(dt.bfloat16)
                    mtT = mtile[:, 64 + KSUP:MW].bitcast(dt.bfloat16)

                    # on-device one-hot membership: mt[p, kk*128+j] =
                    # (dst_slot[p, kk] == j)
                    mt = mtb.tile([128, KSUP * 128], dt.bfloat16, tag="mt")
                    nc.vector.tensor_tensor(
                        out=mt[:].rearrange("p (k j) -> p k j", k=KSUP),
                        in0=iota_row[:].rearrange("p (k j) -> p k j", k=KSUP),
                        in1=dcol[:].unsqueeze(2).to_broadcast(
                            [128, KSUP, 128]),
                        op=Alu.is_equal)

                    gt = gbuf.tile([128, KSUP, tw], dt.bfloat16,
                                   tag=f"g{min(layer, 2)}")
                    for (a, b, tag) in runs_by_sup[sup]:
                        a0, b0 = a - sup * KSUP, b - sup * KSUP
                        nidx = (b - a) * 128
                        nc.gpsimd.dma_gather(
                            out_ap=gt[:, a0:b0, :], in_ap=tabg[tag][:],
                            idxs_ap=mtile[:, a0 * 8:b0 * 8],
                            num_idxs=nidx, num_idxs_reg=nidx, elem_size=tw,
                            queue_num=qctr[0] % 4)
                        qctr[0] += 1

                    # per-edge a_dst via one-hot matmul against the
                    # tile-local a_dst table
                    psw = psum_w.tile([128, KSUP * nh], dt.float32,
                                      tag="psw")
                    for kk in range(KSUP):
                        t = int(tile_of_chunk[sup * KSUP + kk])
                        nc.tensor.matmul(
                            psw[:, kk * nh:(kk + 1) * nh],
                            lhsT=mtT[:, kk * 128:(kk + 1) * 128],
                            rhs=adst_tab[:, t * nh:(t + 1) * nh],
                            start=True, stop=True)

                    wt = gbuf.tile([128, KSUP, nh], dt.bfloat16, tag="wt")
                    nc.vector.tensor_tensor(
                        out=wt[:], in0=gt[:, :, nfeat:nfeat + nh],
                        in1=psw[:].rearrange("p (k h) -> p k h", k=KSUP),
                        op=Alu.add)
                    nc.vector.scalar_tensor_tensor(
                        out=wt[:], in0=wt[:], scalar=NEG_SLOPE, in1=wt[:],
                        op0=Alu.mult, op1=Alu.max)
                    nc.scalar.activation(out=gt[:, :, nfeat:nfeat + nh],
                                         in_=wt[:], func=Act.Exp)
                    if layer != 3:
                        nc.vector.tensor_tensor(
                            out=gt[:, :, :nfeat].rearrange(
                                "p k (c h) -> p k c h", h=HEADS),
                            in0=gt[:, :, :nfeat].rearrange(
                                "p k (c h) -> p k c h", h=HEADS),
                            in1=gt[:, :, nfeat:nfeat + nh].unsqueeze(2)
                            .to_broadcast([128, KSUP, HID, HEADS]),
                            op=Alu.mult)
                    else:
                        nc.vector.tensor_tensor(
                            out=gt[:, :, :nfeat], in0=gt[:, :, :nfeat],
                            in1=gt[:, :, nfeat:nfeat + nh].to_broadcast(
                                [128, KSUP, nfeat]),
                            op=Alu.mult)

                    # prefetch self-loop rows for tiles opening in this
                    # sup; consumed by the (one-sup-deferred) seg stage
                    for kk in range(KSUP):
                        q = sup * KSUP + kk
                        t = int(tile_of_chunk[q])
                        if q == first_chunk[t]:
                            sf = sfbuf.tile([128, gw], dt.bfloat16,
                                            tag="selfread")
                            nc.sync.dma_start(
                                out=sf[:],
                                in_=selfbuf[t * 128:(t + 1) * 128, :gw])
                            sf_map[t] = sf

                    pend.append((sup, gt, mt))
                    if len(pend) >= 2:
                        seg_stage(*pend.pop(0))
                while pend:
                    seg_stage(*pend.pop(0))

            nphase = int(os.environ.get("GAT_PHASES", "3"))
            for _rep in range(repeat):
                tab1g = [dram.tile([NCORES * (b - a) * 128, TW], dt.bfloat16,
                                   name=f"tab1g{i}_{_rep}")
                         for i, (a, b) in enumerate(groups)]
                tab2g = [dram.tile([NCORES * (b - a) * 128, TW], dt.bfloat16,
                                   addr_space="Shared",
                                   name=f"tab2g{i}_{_rep}")
                         for i, (a, b) in enumerate(groups)]
                tab3g = [dram.tile([NCORES * (b - a) * 128, TW3],
                                   dt.bfloat16, addr_space="Shared",
                                   name=f"tab3g{i}_{_rep}")
                         for i, (a, b) in enumerate(groups)]
                h1_phase(tab1g)
                if nphase >= 1:
                    aggregate(1, tab1g, adstA, rhs2_s, b1_s, adstB, selfB,
                              selfA, ag_next=(loc2g, tab2g))
                if nphase >= 2:
                    aggregate(2, tab2g, adstB, rhs3_s, b2_s, None, None,
                              selfB, ag_next=(loc3g, tab3g))
                if nphase >= 3:
                    aggregate(3, tab3g, adstC, None, None, None, None,
                              selfC)

    nc.compile()
    return nc


def _make_in_maps(x, edge_index, W1, as1, ad1, b1, W2, as2, ad2, b2,
                  W3, as3, ad3, b3, g):
    rhs1, rhs2, rhs3, b1r, b2r, b3r = _prep_weights(
        W1, as1, ad1, b1, W2, as2, ad2, b2, W3, as3, ad3, b3)
    x = np.asarray(x, np.float32)
    npc, nmax = g["npc"], g["nmax"]
    xTf = np.zeros((IN_C, NCORES * nmax), _BF16)
    for k in range(NCORES):
        xTf[:, k * nmax:k * nmax + npc] = x[k * npc:(k + 1) * npc].T.astype(
            _BF16)
    in_maps = []
    for k in range(NCORES):
        xT = np.zeros((IN_C, nmax), _BF16)
        xT[:, :npc] = x[k * npc:(k + 1) * npc].T.astype(_BF16)
        in_maps.append({
            "xTf": xTf, "xT": xT, "rhs1": rhs1, "rhs2": rhs2, "rhs3": rhs3,
            "b1r": b1r, "b2r": b2r, "b3r": b3r,
            "meta": g["meta"][k],
        })
    return in_maps


_CACHE = {}


def kernel(x, edge_index, W1, as1, ad1, b1, W2, as2, ad2, b2, W3, as3, ad3, b3,
           _repeat=1):
    from concourse.bass_utils import run_bass_kernel_spmd

    edge_index = np.asarray(edge_index)
    g = _prep_graph(edge_index)

    key = (hash(edge_index.tobytes()), _repeat)
    if key not in _CACHE:
        _CACHE[key] = _build_bass(g, repeat=_repeat)
    nc = _CACHE[key]

    in_maps = _make_in_maps(x, edge_index, W1, as1, ad1, b1,
                            W2, as2, ad2, b2, W3, as3, ad3, b3, g)

    res = run_bass_kernel_spmd(nc, in_maps, core_ids=list(range(NCORES)))
    npc = g["npc"]
    outf = np.zeros((N, OUT_C), np.float32)
    for k in range(NCORES):
        outf[k * npc:(k + 1) * npc] = res.results[k]["out"][:npc]
    return outf


# revision 14
# speedup vs baseline: 1.0475x; 1.0475x over previous
"""3-layer GAT on 8 Trainium2 NeuronCores (Bass/Tile).

Edge-sharded by destination range:
  - Nodes split into 8 contiguous ranges (one per core); each core owns the
    softmax + aggregation for its destination nodes.
  - Layer 1's packed per-node table [h | a_src] (c-major feature order) is
    computed REPLICATED on every core (x is cheap to re-multiply at bf16),
    killing the first AllGather.  Layers 2/3 AllGather their tables in 4
    row-chunks issued as destination-tile groups complete, overlapping the
    collective with the remaining aggregation compute.
  - Edges (with self loops) are bucketed per core into 128-dst tiles x
    128-edge chunks; chunk structure (incl. lo/hi int16-index table halves)
    is made identical across cores so one SPMD instruction stream fits all.
  - Per 8-chunk super-batch the kernel dma_gathers source rows + dest
    attention rows, computes w = exp(leaky_relu(a_src+a_dst)) (softmax
    shift-invariance removes the segment-max pass at these value ranges),
    scales messages by w, and segment-sums with matmuls against one-hot
    membership matrices, keeping numerator and denominator together in
    PSUM.  The membership matrix mt is built ON DEVICE per super-batch with
    a single is_equal broadcast op against host-packed dst-slot columns
    (the transposed variant mtT, needed for the per-edge a_dst matmul,
    stays host-built in meta).  The per-tile epilogue divides, applies
    bias/relu, and feeds the next layer's matmul whose rhs
    [W | W@att_src | W@att_dst] also emits the next attention scores.
"""

import numpy as np
import ml_dtypes

N = 50000
E = 800000
IN_C = 128
HID = 32
OUT_C = 40
HEADS = 8
NEG_SLOPE = 0.2
NCORES = 8

_BF16 = ml_dtypes.bfloat16

KSUP = 8  # chunks per gather super-batch (1024 idx = dma_gather limit)
# AllGather row-chunk tile-group sizes (sum = ntiles = 49). Front-loaded so
# early chunks (issued earliest) carry the bytes and the tail chunk - the
# only one with no compute left to hide under - is tiny.
AG_SIZES = [24, 14, 8, 3]


def _cmajor_perm(heads, ch):
    f_new = np.arange(heads * ch)
    return (f_new % heads) * ch + f_new // heads  # perm[new] = old


def _attn_cols(w, att):
    heads, ch = att.shape
    return np.einsum("khc,hc->kh", w.reshape(-1, heads, ch), att).astype(np.float32)


def _prep_weights(W1, as1, ad1, b1, W2, as2, ad2, b2, W3, as3, ad3, b3):
    W1 = np.asarray(W1, np.float32)
    W2 = np.asarray(W2, np.float32)
    W3 = np.asarray(W3, np.float32)
    perm = _cmajor_perm(HEADS, HID)

    rhs1 = np.concatenate(
        [W1[:, perm], _attn_cols(W1, np.asarray(as1, np.float32)),
         _attn_cols(W1, np.asarray(ad1, np.float32))], axis=1).astype(_BF16)
    W2r = W2[perm, :]
    rhs2 = np.concatenate(
        [W2r[:, perm], _attn_cols(W2r, np.asarray(as2, np.float32)),
         _attn_cols(W2r, np.asarray(ad2, np.float32))], axis=1).astype(np.float32)
    W3r = W3[perm, :]
    as3p = (W3r @ np.asarray(as3, np.float32)[0]).reshape(-1, 1)
    ad3p = (W3r @ np.asarray(ad3, np.float32)[0]).reshape(-1, 1)
    rhs3 = np.concatenate([W3r, as3p, ad3p], axis=1).astype(np.float32)

    def bcast(b):
        return np.tile(np.asarray(b, np.float32)[None, :], (128, 1))

    return (rhs1, rhs2, rhs3,
            bcast(np.asarray(b1, np.float32)[perm]),
            bcast(np.asarray(b2, np.float32)[perm]),
            bcast(np.asarray(b3, np.float32)))


def _ag_groups(ntiles):
    """Tile-index ranges of the AllGather row-chunk groups."""
    sizes = AG_SIZES if sum(AG_SIZES) == ntiles else [ntiles]
    groups = []
    t0 = 0
    for s in sizes:
        groups.append((t0, t0 + s))
        t0 += s
    return groups


def _prep_graph(edge_index):
    """Slot edges into the SPMD-uniform (tile, section, chunk) grid.

    Self loops are NOT materialized as edges; their contribution is
    injected per destination tile in the aggregation prologue matmul.
    Edges are tagged by the AllGather group of their SOURCE row, since
    each group is a separate table tensor (single-writer collectives);
    gather indices are rows within the group tensor.
    """
    src = np.asarray(edge_index[0]).astype(np.int64)
    dst = np.asarray(edge_index[1]).astype(np.int64)

    npc = N // NCORES
    ntiles = (npc + 127) // 128
    nmax = ntiles * 128

    groups = _ag_groups(ntiles)
    G = len(groups)
    grp_of_tile = np.zeros(ntiles, np.int64)
    grp_a = np.zeros(G, np.int64)
    grp_sz = np.zeros(G, np.int64)
    for gi, (a, b) in enumerate(groups):
        grp_of_tile[a:b] = gi
        grp_a[gi] = a
        grp_sz[gi] = b - a

    core_of = dst // npc
    d_loc = dst - core_of * npc
    tile_of = d_loc // 128
    s_core = src // npc
    s_loc = src - s_core * npc
    s_tag = grp_of_tile[s_loc // 128]
    # row within the group tensor [NCORES * grp_sz * 128]
    s_row = (s_core * grp_sz[s_tag] + (s_loc // 128 - grp_a[s_tag])) * 128 \
        + s_loc % 128
    assert s_row.max() < 32768

    cnt = np.zeros((NCORES, ntiles, G), np.int64)
    np.add.at(cnt, (core_of, tile_of, s_tag), 1)
    sec_cpt = np.ceil(cnt / 128).astype(np.int64).max(axis=0)  # [ntiles, G]
    sec_cpt[:, 0] = np.maximum(sec_cpt[:, 0], 1)

    total = int(sec_cpt.sum())
    pad = (-total) % KSUP
    sec_cpt[-1, -1] += pad
    total += pad
    nsup = total // KSUP

    # pair adjacent tiles so same-tag sections are contiguous: longer
    # dma_gather runs (fewer gathers) at the cost of two concurrently
    # open PSUM accumulators
    sec_order = []
    for t0 in range(0, ntiles, 2):
        ts = [t0] if t0 + 1 >= ntiles else [t0, t0 + 1]
        for gtag in range(G):
            sec_order += [(t, gtag) for t in ts]
    tile_of_chunk = []
    tag_of_chunk = []
    sec_base = np.zeros((ntiles, G), np.int64)
    off = 0
    for (t, tg) in sec_order:
        n = int(sec_cpt[t, tg])
        sec_base[t, tg] = off
        tile_of_chunk += [t] * n
        tag_of_chunk += [tg] * n
        off += n
    tile_of_chunk = np.array(tile_of_chunk)
    tag_of_chunk = np.array(tag_of_chunk)

    import os
    mt_dev = os.environ.get("GAT_MT", "dev") == "dev"
    # idx | dst-slot cols (bf16) | mtT [| mts when host-built]
    MW = 64 + KSUP + KSUP * 128 * (1 if mt_dev else 2)
    src_w = np.zeros((NCORES, 128, total * 8), np.int16)
    meta = np.zeros((NCORES, nsup, 128, MW), np.int16)

    order = np.lexsort((src, s_tag, tile_of, core_of))
    src_o = s_row[order]
    dst_o = d_loc[order]
    core_o = core_of[order]
    tile_o = tile_of[order]
    tag_o = s_tag[order]

    for k in range(NCORES):
        m = core_o == k
        t = tile_o[m]
        tg = tag_o[m]
        sr = src_o[m]
        dl = dst_o[m]
        key = t * G + tg
        cnts = np.bincount(key, minlength=ntiles * G)
        st = np.zeros(ntiles * G, np.int64)
        st[1:] = np.cumsum(cnts)[:-1]
        pos = np.arange(len(t)) - st[key]
        q = sec_base[t, tg] + pos // 128
        p = pos % 128
        col = q * 8 + p // 16
        row = p % 16
        for c in range(8):
            src_w[k, row + 16 * c, col] = sr
        D = np.full((total, 128), 255, np.int16)
        D[q, p] = (dl % 128).astype(np.int16)
        # dst-slot columns, bf16 bits: Dcol[s, p, kk] = D[s*8+kk, p]
        Dcol = D.reshape(nsup, KSUP, 128).transpose(0, 2, 1).astype(_BF16)
        meta[k, :, :, 64:64 + KSUP] = Dcol.view(np.int16)
        # host-built transposed one-hot: mtT[chunk][j, p] = (dl[p] == j)
        oneh = (D[:, :, None] == np.arange(128, dtype=np.int16)[None, None, :]
                ).astype(_BF16)
        mtTs = oneh.transpose(0, 2, 1).reshape(
            nsup, KSUP, 128, 128).transpose(0, 2, 1, 3).reshape(
            nsup, 128, KSUP * 128)
        meta[k, :, :, 64 + KSUP:64 + KSUP + KSUP * 128] = mtTs.view(np.int16)
        if not mt_dev:
            mts = oneh.reshape(nsup, KSUP, 128, 128).transpose(
                0, 2, 1, 3).reshape(nsup, 128, KSUP * 128)
            meta[k, :, :, 64 + KSUP + KSUP * 128:] = mts.view(np.int16)

    runs = []  # (sup, chunk_lo, chunk_hi, tag)
    for s in range(nsup):
        q0 = s * KSUP
        r0 = q0
        for q in range(q0 + 1, q0 + KSUP + 1):
            if q == q0 + KSUP or tag_of_chunk[q] != tag_of_chunk[r0]:
                runs.append((s, r0, q, int(tag_of_chunk[r0])))
                r0 = q

    for k in range(NCORES):
        meta[k, :, :, :64] = src_w[k].reshape(128, nsup, 64).transpose(
            1, 0, 2)

    return dict(
        meta=meta,
        tile_of_chunk=tile_of_chunk, runs=runs, nsup=nsup, total=total,
        ntiles=ntiles, nmax=nmax, npc=npc,
    )


def _build_bass(g, repeat=1):
    import concourse.bacc as bacc
    import concourse.mybir as mybir
    import concourse.tile as tile
    from concourse.masks import make_identity

    dt = mybir.dt
    Alu = mybir.AluOpType
    Act = mybir.ActivationFunctionType

    ntiles, nmax, nsup, total = g["ntiles"], g["nmax"], g["nsup"], g["total"]
    tile_of_chunk = g["tile_of_chunk"]
    H2 = HEADS * HID  # 256
    PACK = H2 + 2 * HEADS  # 272 psum width: h + a_src + a_dst
    TW = 384  # table row width (768B)
    TW3 = 128  # layer-3 / a_dst table row width (256B)
    GW = H2 + HEADS  # 264 useful gathered cols
    GW3 = OUT_C + 1  # 41
    GTILES = NCORES * ntiles  # replicated layer-1 tiles
    _MT_DEV = os.environ.get("GAT_MT", "dev") == "dev"
    MW = 64 + KSUP + KSUP * 128 * (1 if _MT_DEV else 2)

    first_chunk = {}
    last_chunk = {}
    for q, t in enumerate(tile_of_chunk):
        first_chunk.setdefault(int(t), q)
        last_chunk[int(t)] = q
    runs_by_sup = {}
    for (s, a, b, tag) in g["runs"]:
        runs_by_sup.setdefault(s, []).append((a, b, tag))

    groups = _ag_groups(ntiles)
    grp_of_tile = {}
    for gi, (a, b) in enumerate(groups):
        for t in range(a, b):
            grp_of_tile[t] = gi

    nc = bacc.Bacc("TRN2", target_bir_lowering=False, debug=False,
                   num_devices=NCORES, num_swdge_queues=4)

    xTf = nc.dram_tensor("xTf", [IN_C, GTILES * 128], dt.bfloat16,
                         kind="ExternalInput")  # full graph, replicated
    xT = nc.dram_tensor("xT", [IN_C, nmax], dt.bfloat16,
                        kind="ExternalInput")  # own slice
    rhs1 = nc.dram_tensor("rhs1", [IN_C, PACK], dt.bfloat16,
                          kind="ExternalInput")
    rhs2 = nc.dram_tensor("rhs2", [H2, PACK], dt.float32, kind="ExternalInput")
    rhs3 = nc.dram_tensor("rhs3", [H2, OUT_C + 2], dt.float32,
                          kind="ExternalInput")
    b1r = nc.dram_tensor("b1r", [128, H2], dt.float32, kind="ExternalInput")
    b2r = nc.dram_tensor("b2r", [128, H2], dt.float32, kind="ExternalInput")
    b3r = nc.dram_tensor("b3r", [128, OUT_C], dt.float32, kind="ExternalInput")
    meta_in = nc.dram_tensor("meta", [nsup, 128, MW],
                             dt.int16, kind="ExternalInput")
    out = nc.dram_tensor("out", [nmax, OUT_C], dt.float32,
                         kind="ExternalOutput")

    import os
    _SIM = bool(int(os.environ.get("GAT_SIM", "0")))

    with tile.TileContext(nc) as tc:
        with (
            tc.tile_pool(name="const", bufs=1) as constp,
            tc.tile_pool(name="sbuf", bufs=6) as sbuf,
            tc.tile_pool(name="gbuf", bufs=8) as gbuf,
            tc.tile_pool(name="mtb", bufs=3) as mtb,
            tc.tile_pool(name="xb", bufs=3) as xb,
            tc.tile_pool(name="epil", bufs=2) as epil,
            tc.tile_pool(name="sfbuf", bufs=4) as sfbuf,
            tc.tile_pool(name="psum_seg", bufs=3, space="PSUM") as psum_seg,
            tc.tile_pool(name="psum_h", bufs=2, space="PSUM") as psum_h,
            tc.tile_pool(name="psum_tp", bufs=1, space="PSUM") as psum_tp,
            tc.tile_pool(name="psum_w", bufs=2, space="PSUM") as psum_w,
            tc.tile_pool(name="dram", bufs=1, space="DRAM") as dram,
        ):
            # ---- constants ----
            xT_s = constp.tile([IN_C, nmax], dt.bfloat16)
            nc.sync.dma_start(out=xT_s[:], in_=xT[:])
            rhs1_s = constp.tile([IN_C, PACK], dt.bfloat16)
            nc.sync.dma_start(out=rhs1_s[:], in_=rhs1[:])
            rhs2_s = constp.tile([128, 2 * PACK], dt.float32)
            nc.sync.dma_start(
                out=rhs2_s[:].rearrange("p (k f) -> p k f", k=2),
                in_=rhs2[:].rearrange("(k p) f -> p k f", p=128))
            rhs3_s = constp.tile([128, 2 * (OUT_C + 2)], dt.float32)
            nc.sync.dma_start(
                out=rhs3_s[:].rearrange("p (k f) -> p k f", k=2),
                in_=rhs3[:].rearrange("(k p) f -> p k f", p=128))
            b1_s = constp.tile([128, H2], dt.float32)
            nc.sync.dma_start(out=b1_s[:], in_=b1r[:])
            b2_s = constp.tile([128, H2], dt.float32)
            nc.sync.dma_start(out=b2_s[:], in_=b2r[:])
            b3_s = constp.tile([128, OUT_C], dt.float32)
            nc.sync.dma_start(out=b3_s[:], in_=b3r[:])
            ident = constp.tile([128, 128], dt.float32)
            make_identity(nc, ident[:])
            identb = constp.tile([128, 128], dt.bfloat16)
            make_identity(nc, identb[:])
            zpad = constp.tile([128, TW3 - 1], dt.bfloat16)
            nc.vector.memset(zpad[:], 0.0)
            # tiled iota row: iota_row[p, kk*128 + j] = j  (bf16, exact)
            iota_row = constp.tile([128, KSUP * 128], dt.bfloat16)
            nc.gpsimd.iota(iota_row[:], pattern=[[0, KSUP], [1, 128]],
                           channel_multiplier=0,
                           allow_small_or_imprecise_dtypes=True)

            # per-layer destination-attention tables (tile-local, SBUF)
            adstA = constp.tile([128, ntiles * HEADS], dt.bfloat16)
            adstB = constp.tile([128, ntiles * HEADS], dt.bfloat16)
            adstC = constp.tile([128, ntiles], dt.bfloat16)

            # ---- DRAM temporaries ----
            # per-AG-chunk local slices of the layer-2/3 tables
            loc2g = [dram.tile([(b - a) * 128, TW], dt.bfloat16,
                               name=f"loc2g{i}")
                     for i, (a, b) in enumerate(groups)]
            loc3g = [dram.tile([(b - a) * 128, TW3], dt.bfloat16,
                               name=f"loc3g{i}")
                     for i, (a, b) in enumerate(groups)]
            # per-layer self-loop contributions [w*h | w], injected as the
            # accumulation-starting matmul of each destination tile
            selfA = dram.tile([nmax, GW], dt.bfloat16)
            selfB = dram.tile([nmax, GW], dt.bfloat16)
            selfC = dram.tile([nmax, GW3], dt.bfloat16)

            # zero never-written pad columns once (NaN hygiene)
            for gi, (a, b) in enumerate(groups):
                nt = b - a
                for buf, c0 in ((loc2g[gi], GW), (loc3g[gi], GW3)):
                    w = buf.shape[1] - c0
                    nc.sync.dma_start(
                        out=buf[:].rearrange("(t p) w -> p t w", p=128)[
                            :, :, c0:],
                        in_=zpad[:, :w].unsqueeze(1).to_broadcast(
                            [128, nt, w]))

            def loc2_write(t):
                gi = grp_of_tile[t]
                return loc2g[gi], (t - groups[gi][0]) * 128

            def loc3_write(t):
                gi = grp_of_tile[t]
                return loc3g[gi], (t - groups[gi][0]) * 128

            def extras12(ps, pk, adst_next, self_next, t):
                """a_dst table entry + self-loop row for own tile t.

                ps: PSUM [128, PACK]; pk: packed bf16 [128, GW] of same."""
                nc.scalar.copy(
                    out=adst_next[:, t * HEADS:(t + 1) * HEADS],
                    in_=ps[:, GW:GW + HEADS])
                # self-loop term for the next aggregation: [w*h | w],
                # w = exp(leaky(a_src + a_dst)) of the node itself
                ws = epil.tile([128, HEADS], dt.bfloat16, tag="ws")
                nc.vector.tensor_tensor(
                    out=ws[:], in0=pk[:, H2:GW],
                    in1=adst_next[:, t * HEADS:(t + 1) * HEADS], op=Alu.add)
                nc.vector.scalar_tensor_tensor(
                    out=ws[:], in0=ws[:], scalar=NEG_SLOPE, in1=ws[:],
                    op0=Alu.mult, op1=Alu.max)
                nc.scalar.activation(out=ws[:], in_=ws[:], func=Act.Exp)
                sr = epil.tile([128, GW], dt.bfloat16, tag="selfr")
                nc.vector.tensor_tensor(
                    out=sr[:, :H2].rearrange("p (c h) -> p c h", h=HEADS),
                    in0=ps[:, :H2].rearrange("p (c h) -> p c h", h=HEADS),
                    in1=ws[:].unsqueeze(1).to_broadcast([128, HID, HEADS]),
                    op=Alu.mult)
                nc.vector.tensor_copy(sr[:, H2:GW], ws[:])
                nc.sync.dma_start(
                    out=self_next[t * 128:(t + 1) * 128, :GW], in_=sr[:])

            def pack12(ps, adst_next, self_next, t):
                """Layer-1/2 epilogue tail for own tile t: write local table
                slice + extras."""
                pk = epil.tile([128, GW], dt.bfloat16, tag="pack")
                nc.scalar.copy(out=pk[:], in_=ps[:, :GW])
                buf, r0 = loc2_write(t)
                nc.sync.dma_start(out=buf[r0:r0 + 128, :GW], in_=pk[:])
                extras12(ps, pk, adst_next, self_next, t)

            def pack3(ps, t):
                pk = epil.tile([128, GW3], dt.bfloat16, tag="pack")
                nc.scalar.copy(out=pk[:], in_=ps[:, :GW3])
                buf, r0 = loc3_write(t)
                nc.sync.dma_start(out=buf[r0:r0 + 128, :GW3], in_=pk[:])
                nc.scalar.copy(out=adstC[:, t:t + 1],
                               in_=ps[:, GW3:GW3 + 1])
                ws = epil.tile([128, 1], dt.float32, tag="ws3")
                nc.vector.tensor_tensor(
                    out=ws[:], in0=pk[:, OUT_C:OUT_C + 1],
                    in1=adstC[:, t:t + 1], op=Alu.add)
                nc.vector.scalar_tensor_tensor(
                    out=ws[:], in0=ws[:], scalar=NEG_SLOPE, in1=ws[:],
                    op0=Alu.mult, op1=Alu.max)
                nc.scalar.activation(out=ws[:], in_=ws[:], func=Act.Exp)
                sr = epil.tile([128, GW3], dt.bfloat16, tag="selfr3")
                nc.vector.tensor_scalar(
                    out=sr[:, :OUT_C], in0=ps[:, :OUT_C],
                    scalar1=ws[:, :1], scalar2=None, op0=Alu.mult)
                nc.vector.tensor_copy(sr[:, OUT_C:GW3], ws[:])
                nc.sync.dma_start(
                    out=selfC[t * 128:(t + 1) * 128, :GW3], in_=sr[:])

            def h1_phase(tab1g):
                # pass 1: own-slice extras (a_dst table + self rows)
                for t in range(ntiles):
                    ps = psum_h.tile([128, PACK], dt.float32, tag="hps")
                    nc.tensor.matmul(
                        ps[:], lhsT=xT_s[:, t * 128:(t + 1) * 128],
                        rhs=rhs1_s[:], start=True, stop=True)
                    pk = epil.tile([128, GW], dt.bfloat16, tag="pack")
                    nc.scalar.copy(out=pk[:], in_=ps[:, :GW])
                    extras12(ps, pk, adstA, selfA, t)
                # pass 2: replicated full-graph table build (no AllGather)
                XS = 8  # tiles per x super-load
                for ss in range((GTILES + XS - 1) // XS):
                    t0 = ss * XS
                    nt = min(XS, GTILES - t0)
                    xt = xb.tile([IN_C, XS * 128], dt.bfloat16, tag="xt")
                    nc.sync.dma_start(
                        out=xt[:, :nt * 128],
                        in_=xTf[:, t0 * 128:(t0 + nt) * 128])
                    for j in range(nt):
                        gt_i = t0 + j
                        k, t = gt_i // ntiles, gt_i % ntiles
                        gi = grp_of_tile[t]
                        a, b = groups[gi]
                        r0 = (k * (b - a) + (t - a)) * 128
                        ps = psum_h.tile([128, PACK], dt.float32, tag="hps")
                        nc.tensor.matmul(
                            ps[:, :GW], lhsT=xt[:, j * 128:(j + 1) * 128],
                            rhs=rhs1_s[:, :GW], start=True, stop=True)
                        pk = epil.tile([128, GW], dt.bfloat16, tag="pack")
                        nc.scalar.copy(out=pk[:], in_=ps[:, :GW])
                        nc.sync.dma_start(
                            out=tab1g[gi][r0:r0 + 128, :GW], in_=pk[:])

            def allgather_chunk(locg, tabg, gi):
                if _SIM:
                    # TimelineSim can't model collectives: stand in a local
                    # DMA copy so deps stay realistic.
                    nc.sync.dma_start(out=tabg[gi][:locg[gi].shape[0]],
                                      in_=locg[gi][:])
                    return
                nc.gpsimd.collective_compute(
                    "AllGather", Alu.bypass,
                    replica_groups=[list(range(NCORES))],
                    ins=[locg[gi][:].opt()], outs=[tabg[gi][:].opt()])

            def epilogue12(t, ps, rhs_next_s, b_s, layer, adst_next,
                           self_next):
                deneps = epil.tile([128, HEADS], dt.float32, tag="deneps")
                nc.vector.tensor_scalar_add(deneps[:], ps[:, H2:H2 + HEADS],
                                            1e-16)
                recip = epil.tile([128, HEADS], dt.float32, tag="recip")
                nc.vector.reciprocal(recip[:], deneps[:])
                act = epil.tile([128, H2], dt.float32, tag="act")
                nc.vector.tensor_tensor(
                    out=act[:].rearrange("p (c h) -> p c h", h=HEADS),
                    in0=ps[:, :H2].rearrange("p (c h) -> p c h", h=HEADS),
                    in1=recip[:].unsqueeze(1).to_broadcast([128, HID, HEADS]),
                    op=Alu.mult)
                nc.vector.tensor_add(out=act[:], in0=act[:], in1=b_s[:])
                nc.scalar.activation(out=act[:], in_=act[:], func=Act.Relu)
                w = PACK if layer == 1 else OUT_C + 2
                hps = psum_h.tile([128, PACK], dt.float32, tag="hps")
                for kc in range(2):
                    tp = psum_tp.tile([128, 128], dt.float32, tag="tp")
                    nc.tensor.transpose(
                        out=tp[:], in_=act[:, kc * 128:(kc + 1) * 128],
                        identity=ident[:])
                    aT = epil.tile([128, 128], dt.float32, tag="aT")
                    nc.scalar.copy(out=aT[:], in_=tp[:])
                    nc.tensor.matmul(
                        hps[:, :w], lhsT=aT[:],
                        rhs=rhs_next_s[:, kc * w:(kc + 1) * w],
                        start=(kc == 0), stop=(kc == 1))
                if layer == 1:
                    pack12(hps, adst_next, self_next, t)
                else:
                    pack3(hps, t)

            def epilogue3(t, ps):
                deneps = epil.tile([128, 1], dt.float32, tag="deneps3")
                nc.vector.tensor_scalar_add(deneps[:], ps[:, OUT_C:OUT_C + 1],
                                            1e-16)
                recip = epil.tile([128, 1], dt.float32, tag="recip3")
                nc.vector.reciprocal(recip[:], deneps[:])
                o3 = epil.tile([128, OUT_C], dt.float32, tag="o3")
                nc.vector.tensor_scalar(
                    out=o3[:], in0=ps[:, :OUT_C], scalar1=recip[:, :1],
                    scalar2=None, op0=Alu.mult)
                nc.vector.tensor_add(out=o3[:], in0=o3[:], in1=b3_s[:])
                mneg = epil.tile([128, 1], dt.float32, tag="mneg")
                nc.vector.tensor_reduce(
                    out=mneg[:], in_=o3[:], axis=mybir.AxisListType.X,
                    op=Alu.max, negate=True)
                es = epil.tile([128, OUT_C], dt.float32, tag="es")
                ssum = epil.tile([128, 1], dt.float32, tag="ssum")
                nc.scalar.activation(out=es[:], in_=o3[:], func=Act.Exp,
                                     bias=mneg[:, :1], accum_out=ssum[:, :1])
                lse = epil.tile([128, 1], dt.float32, tag="lse")
                nc.scalar.activation(out=lse[:], in_=ssum[:], func=Act.Ln)
                fin = epil.tile([128, OUT_C], dt.float32, tag="fin")
                nc.vector.tensor_scalar(
                    out=fin[:], in0=o3[:], scalar1=mneg[:, :1],
                    scalar2=lse[:, :1], op0=Alu.add, op1=Alu.subtract)
                nc.sync.dma_start(out=out[t * 128:(t + 1) * 128, :], in_=fin[:])

            def aggregate(layer, tabg, adst_tab, rhs_next_s, b_s,
                          adst_next, self_next, selfbuf, ag_next=None):
                """ag_next: (loc_list, next_table_list) to AllGather in
                chunks as tile groups complete (layer 1 -> tab2,
                layer 2 -> tab3)."""
                if layer == 3:
                    gw, nfeat, nh, tw = GW3, OUT_C, 1, TW3
                else:
                    gw, nfeat, nh, tw = GW, H2, HEADS, TW

                ps_open = {}
                sf_map = {}
                pend = []
                qctr = [0]

                def seg_stage(sup, gt, mt):
                    for kk in range(KSUP):
                        q = sup * KSUP + kk
                        t = int(tile_of_chunk[q])
                        if q == first_chunk[t]:
                            ps_new = psum_seg.tile([128, GW], dt.float32,
                                                   tag="segps")
                            ps_open[t] = ps_new
                            nc.tensor.matmul(
                                ps_new[:, :gw], lhsT=identb[:],
                                rhs=sf_map.pop(t)[:], start=True,
                                stop=False)
                        nc.tensor.matmul(
                            ps_open[t][:, :gw],
                            lhsT=mt[:, kk * 128:(kk + 1) * 128],
                            rhs=gt[:, kk, :gw],
                            start=False,
                            stop=(q == last_chunk[t]))
                        if q == last_chunk[t]:
                            ps_done = ps_open.pop(t)
                            if layer == 3:
                                epilogue3(t, ps_done)
                            else:
                                epilogue12(t, ps_done, rhs_next_s, b_s,
                                           layer, adst_next, self_next)
                            if ag_next is not None:
                                locs, ntabg = ag_next
                                gi = grp_of_tile[t]
                                if t == groups[gi][1] - 1:
                                    allgather_chunk(locs, ntabg, gi)

                for sup in range(nsup):
                    mtile = sbuf.tile([128, MW], dt.int16, tag="meta")
                    nc.sync.dma_start(out=mtile[:], in_=meta_in[sup])
                    dcol = mtile[:, 64:64 + KSUP].bitcast(dt.bfloat16)
                    mtT = mtile[:, 64 + KSUP:MW].bitcast(dt.bfloat16)

                    # on-device one-hot membership: mt[p, kk*128+j] =
                    # (dst_slot[p, kk] == j)
                    mt = mtb.tile([128, KSUP * 128], dt.bfloat16, tag="mt")
                    nc.vector.tensor_tensor(
                        out=mt[:].rearrange("p (k j) -> p k j", k=KSUP),
                        in0=iota_row[:].rearrange("p (k j) -> p k j", k=KSUP),
                        in1=dcol[:].unsqueeze(2).to_broadcast(
                            [128, KSUP, 128]),
                        op=Alu.is_equal)

                    gt = gbuf.tile([128, KSUP, tw], dt.bfloat16,
                                   tag=f"g{min(layer, 2)}")
                    for (a, b, tag) in runs_by_sup[sup]:
                        a0, b0 = a - sup * KSUP, b - sup * KSUP
                        nidx = (b - a) * 128
                        nc.gpsimd.dma_gather(
                            out_ap=gt[:, a0:b0, :], in_ap=tabg[tag][:],
                            idxs_ap=mtile[:, a0 * 8:b0 * 8],
                            num_idxs=nidx, num_idxs_reg=nidx, elem_size=tw,
                            queue_num=qctr[0] % 4)
                        qctr[0] += 1

                    # per-edge a_dst via one-hot matmul against the
                    # tile-local a_dst table
                    psw = psum_w.tile([128, KSUP * nh], dt.float32,
                                      tag="psw")
                    for kk in range(KSUP):
                        t = int(tile_of_chunk[sup * KSUP + kk])
                        nc.tensor.matmul(
                            psw[:, kk * nh:(kk + 1) * nh],
                            lhsT=mtT[:, kk * 128:(kk + 1) * 128],
                            rhs=adst_tab[:, t * nh:(t + 1) * nh],
                            start=True, stop=True)

                    wt = gbuf.tile([128, KSUP, nh], dt.bfloat16, tag="wt")
                    nc.vector.tensor_tensor(
                        out=wt[:], in0=gt[:, :, nfeat:nfeat + nh],
                        in1=psw[:].rearrange("p (k h) -> p k h", k=KSUP),
                        op=Alu.add)
                    nc.vector.scalar_tensor_tensor(
                        out=wt[:], in0=wt[:], scalar=NEG_SLOPE, in1=wt[:],
                        op0=Alu.mult, op1=Alu.max)
                    nc.scalar.activation(out=gt[:, :, nfeat:nfeat + nh],
                                         in_=wt[:], func=Act.Exp)
                    if layer != 3:
                        nc.vector.tensor_tensor(
                            out=gt[:, :, :nfeat].rearrange(
                                "p k (c h) -> p k c h", h=HEADS),
                            in0=gt[:, :, :nfeat].rearrange(
                                "p k (c h) -> p k c h", h=HEADS),
                            in1=gt[:, :, nfeat:nfeat + nh].unsqueeze(2)
                            .to_broadcast([128, KSUP, HID, HEADS]),
                            op=Alu.mult)
                    else:
                        nc.vector.tensor_tensor(
                            out=gt[:, :, :nfeat], in0=gt[:, :, :nfeat],
                            in1=gt[:, :, nfeat:nfeat + nh].to_broadcast(
                                [128, KSUP, nfeat]),
                            op=Alu.mult)

                    # prefetch self-loop rows for tiles opening in this
                    # sup; consumed by the (one-sup-deferred) seg stage
                    for kk in range(KSUP):
                        q = sup * KSUP + kk
                        t = int(tile_of_chunk[q])
                        if q == first_chunk[t]:
                            sf = sfbuf.tile([128, gw], dt.bfloat16,
                                            tag="selfread")
                            nc.sync.dma_start(
                                out=sf[:],
                                in_=selfbuf[t * 128:(t + 1) * 128, :gw])
                            sf_map[t] = sf

                    pend.append((sup, gt, mt))
                    if len(pend) >= 2:
                        seg_stage(*pend.pop(0))
                while pend:
                    seg_stage(*pend.pop(0))

            nphase = int(os.environ.get("GAT_PHASES", "3"))
            for _rep in range(repeat):
                tab1g = [dram.tile([NCORES * (b - a) * 128, TW], dt.bfloat16,
                                   name=f"tab1g{i}_{_rep}")
                         for i, (a, b) in enumerate(groups)]
                tab2g = [dram.tile([NCORES * (b - a) * 128, TW], dt.bfloat16,
                                   addr_space="Shared",
                                   name=f"tab2g{i}_{_rep}")
                         for i, (a, b) in enumerate(groups)]
                tab3g = [dram.tile([NCORES * (b - a) * 128, TW3],
                                   dt.bfloat16, addr_space="Shared",
                                   name=f"tab3g{i}_{_rep}")
                         for i, (a, b) in enumerate(groups)]
                h1_phase(tab1g)
                if nphase >= 1:
                    aggregate(1, tab1g, adstA, rhs2_s, b1_s, adstB, selfB,
                              selfA, ag_next=(loc2g, tab2g))
                if nphase >= 2:
                    aggregate(2, tab2g, adstB, rhs3_s, b2_s, None, None,
                              selfB, ag_next=(loc3g, tab3g))
                if nphase >= 3:
                    aggregate(3, tab3g, adstC, None, None, None, None,
                              selfC)

    nc.compile()
    return nc


def _make_in_maps(x, edge_index, W1, as1, ad1, b1, W2, as2, ad2, b2,
                  W3, as3, ad3, b3, g):
    rhs1, rhs2, rhs3, b1r, b2r, b3r = _prep_weights(
        W1, as1, ad1, b1, W2, as2, ad2, b2, W3, as3, ad3, b3)
    x = np.asarray(x, np.float32)
    npc, nmax = g["npc"], g["nmax"]
    xTf = np.zeros((IN_C, NCORES * nmax), _BF16)
    for k in range(NCORES):
        xTf[:, k * nmax:k * nmax + npc] = x[k * npc:(k + 1) * npc].T.astype(
            _BF16)
    in_maps = []
    for k in range(NCORES):
        xT = np.zeros((IN_C, nmax), _BF16)
        xT[:, :npc] = x[k * npc:(k + 1) * npc].T.astype(_BF16)
        in_maps.append({
            "xTf": xTf, "xT": xT, "rhs1": rhs1, "rhs2": rhs2, "rhs3": rhs3,
            "b1r": b1r, "b2r": b2r, "b3r": b3r,
            "meta": g["meta"][k],
        })
    return in_maps


_CACHE = {}


def kernel(x, edge_index, W1, as1, ad1, b1, W2, as2, ad2, b2, W3, as3, ad3, b3,
           _repeat=1):
    from concourse.bass_utils import run_bass_kernel_spmd

    edge_index = np.asarray(edge_index)
    g = _prep_graph(edge_index)

    key = (hash(edge_index.tobytes()), _repeat)
    if key not in _CACHE:
        _CACHE[key] = _build_bass(g, repeat=_repeat)
    nc = _CACHE[key]

    in_maps = _make_in_maps(x, edge_index, W1, as1, ad1, b1,
                            W2, as2, ad2, b2, W3, as3, ad3, b3, g)

    res = run_bass_kernel_spmd(nc, in_maps, core_ids=list(range(NCORES)))
    npc = g["npc"]
    outf = np.zeros((N, OUT_C), np.float32)
    for k in range(NCORES):
        outf[k * npc:(k + 1) * npc] = res.results[k]["out"][:npc]
    return outf
